# revision 8
# baseline (speedup 1.0000x reference)
"""ArcFace (AngularPenaltySMLoss) distributed Trainium2 kernel, v4.

Strategy (tensor-parallel over classes, per the sharding hint):
  - Shard W's C=100000 rows over 8 cores (12500 each).
  - Host: normalize x; pre-scale and cast x, W to fp8e4m3; lay both out
    chunk-contiguously so every DMA is 128 straight partition lines.
  - Device (SPMD, no collectives): per (chunk, b-tile) unit, fp8
    DoubleRow matmuls fill a [128, w] PSUM tile (si-outer / k-inner so
    column slices complete progressively). Each tile's columns are then
    consumed split across two engines, ratio ~61/39 so both hide under
    the PE stream:
      * cols [0:ca]  -> ACT: exp(2*raw) in place + accum_out (free-dim
        sum straight into an accumulator slot).
      * cols [ca:w]  -> DVE: Schraudolph bit-trick exp — tensor_scalar
        affine fp32->int16 (bits of bf16 exp), then one
        scalar_tensor_tensor fold-add over the bitcast-bf16 halves with
        accum_out (fp32).
  - The urgent transfers (x tiles + first W chunk) ride their own DMA
    queue so the 6.4MB W stream can't starve them.
  - Final per-bt reduce of the accumulator slots + [128, 8] DMA out.
  - Host: sum partials over cores, compute the tiny per-sample target /
    arccos / log path in f64, return the scalar loss.
"""

import sys

if "/opt/trn_rl_repo" not in sys.path:
    sys.path.insert(0, "/opt/trn_rl_repo")

import ml_dtypes
import numpy as np

import concourse.bass as bass
import concourse.mybir as mybir
from concourse import bacc
from concourse.bass_utils import run_bass_kernel_spmd
from concourse.tile import TileContext

B, C, D = 1024, 100000, 512
S_SCALE, MARGIN, EPS = 64.0, 0.5, 1e-7
N_CORES = 8
C_SHARD = C // N_CORES          # 12500
P = 128
KO = D // P                     # 4 k-chunks of 128
B_TILES = B // P                # 8
MM_N = 512                      # one matmul output <= one PSUM bank
N_WARM = 4                      # PE warm-up matmuls (bridge DMA fill + HAM)

WSCALE, XSCALE = 8.0, 4.0       # fp8 pre-scales (folded out via ACT_SCALE)
ACT_SCALE = S_SCALE / (WSCALE * XSCALE)   # 2.0

# Schraudolph bf16 exp bits: i16 = rint(A * raw + Badd); bitcast bf16.
# A = ACT_SCALE * 2^7/ln2; Badd = 127*2^7 - C with C calibrated to zero
# the mean relative error for s*logit ~ N(0, 1.28).
SCH_A = ACT_SCALE * 184.66496580927726
SCH_B = 16256.0 - 7.4

# (width, act_cols): per-chunk split of columns between ACT-exp and
# DVE-Schraudolph, balancing measured instruction costs:
#   ACT = 0.833*ca + 576ns   DVE = 1.45*cs + 444ns   PE fill = 0.84*w
CHUNK_SPEC = [
    (512, 312),
    (1748, 1052),
    (2048, 1244),
    (2048, 1244),
    (2048, 1244),
    (2048, 1244),
    (2048, 1244),
]
assert sum(w for w, _ in CHUNK_SPEC) == C_SHARD
N_CHUNKS = len(CHUNK_SPEC)

LAST_RESULT = None
_NC_CACHE = None


def _build_bass():
    nc = bacc.Bacc("TRN2")
    xnt = nc.declare_dram_parameter("xnt", [P, KO * B], mybir.dt.float8e4, isOutput=False)
    wt = nc.declare_dram_parameter("wt", [P, KO * C_SHARD], mybir.dt.float8e4, isOutput=False)
    out = nc.declare_dram_parameter("out", [P, B_TILES], mybir.dt.float32, isOutput=True)

    fp8 = mybir.dt.float8e4
    f32 = mybir.dt.float32
    bf16 = mybir.dt.bfloat16
    i16 = mybir.dt.int16
    DR = mybir.MatmulPerfMode.DoubleRow
    EXP = mybir.ActivationFunctionType.Exp

    with TileContext(nc) as tc:
        with (
            tc.tile_pool(name="xp", bufs=1) as xp,
            tc.tile_pool(name="wp", bufs=1) as wp,
            tc.tile_pool(name="ip", bufs=3) as ip,
            tc.tile_pool(name="ep", bufs=2) as ep,
            tc.tile_pool(name="fp", bufs=2) as fpool,
            tc.tile_pool(name="ac", bufs=1) as ac,
            tc.tile_pool(name="ps", bufs=2, space="PSUM") as psp,
        ):
            # urgent queue (sync): x tiles + first W chunk
            xa = xp.tile([P, 2, B], fp8)
            xb = xp.tile([P, 2, B], fp8)
            nc.sync.dma_start(xa[:], xnt[:, : 2 * B])
            nc.sync.dma_start(xb[:], xnt[:, 2 * B :])

            wts = []
            c0 = 0
            for ci, (cw, _) in enumerate(CHUNK_SPEC):
                t = wp.tile([P, KO, cw], fp8, tag=f"wt{ci}")
                # big W stream rides the idle GpSimd engine's queue so it
                # can't delay the ACT table load or starve the x tiles
                q = nc.sync if ci == 0 else nc.gpsimd
                q.dma_start(t[:], wt[:, 4 * c0 : 4 * (c0 + cw)])
                wts.append(t)
                c0 += cw

            # ACT table warm-up: a tiny exp before any real work so the
            # ~2.7us PSEUDO_LOAD_ACT_FUNC_SET runs during the DMA fill.
            jt = xp.tile([P, 8], f32)
            nc.vector.memset(jt[:], 0.0)
            ja = xp.tile([P, 8], bf16)
            nc.scalar.activation(ja[:], jt[:], EXP)

            # PE warm-up: bridge from engine start to the first
            # data-dependent matmul so HAM un-throttles (~3.4us window).
            wsrc = xp.tile([P, MM_N], fp8, tag="warm_src")
            nc.vector.memset(wsrc[:], 1)
            for _ in range(N_WARM):
                pw = psp.tile([P, 2048], f32, tag="ps")
                nc.tensor.matmul(
                    pw[:, :MM_N], wsrc[:, :P], wsrc[:], start=True, stop=True
                )

            # accumulator: 2 slots (ACT half, DVE half) per (bt, chunk)
            acc = ac.tile([P, B_TILES, 2 * N_CHUNKS], f32)
            out_sb = ac.tile([P, B_TILES], f32)

            for ci, (cw, ca) in enumerate(CHUNK_SPEC):
                wt_t = wts[ci]
                cs = cw - ca
                h = cs // 2
                n_sub = (cw + MM_N - 1) // MM_N
                for bt in range(B_TILES):
                    ps = psp.tile([P, 2048], f32, tag="ps")
                    # si-outer / k-inner: column slices complete
                    # progressively so consumers can start early.
                    for si in range(n_sub):
                        s0 = si * MM_N
                        sw = min(MM_N, cw - s0)
                        for k in (0, 2):
                            nc.tensor.matmul(
                                ps[:, s0 : s0 + sw],
                                (xa if k == 0 else xb)[:, :, bt * P : (bt + 1) * P],
                                wt_t[:, k : k + 2, s0 : s0 + sw],
                                start=(k == 0),
                                stop=(k == 2),
                                perf_mode=DR,
                            )
                    # ACT cols: exp + free-dim accumulate. The exp output
                    # itself is dead — it goes to a recycled SBUF dump
                    # tile rather than back into PSUM, so the DVE's read
                    # of the other columns isn't serialized behind an
                    # in-place PSUM write (Tile tracks deps per tile).
                    exd = ep.tile([P, 1280], bf16, tag="exd")
                    nc.scalar.activation(
                        exd[:, :ca],
                        ps[:, :ca],
                        EXP,
                        scale=ACT_SCALE,
                        accum_out=acc[:, bt, 2 * ci : 2 * ci + 1],
                    )
                    # DVE cols: Schraudolph exp bits + fused fold+accum
                    it = ip.tile([P, 1024], i16, tag="it")
                    nc.vector.tensor_scalar(
                        it[:, :cs],
                        ps[:, ca:cw],
                        SCH_A,
                        SCH_B,
                        mybir.AluOpType.mult,
                        mybir.AluOpType.add,
                    )
                    fo = fpool.tile([P, 512], bf16, tag="fo")
                    nc.vector.scalar_tensor_tensor(
                        fo[:, :h],
                        it[:, 0:h].bitcast(bf16),
                        1.0,
                        it[:, h:cs].bitcast(bf16),
                        mybir.AluOpType.mult,
                        mybir.AluOpType.add,
                        accum_out=acc[:, bt, 2 * ci + 1 : 2 * ci + 2],
                    )

            for bt in range(B_TILES):
                nc.vector.reduce_sum(
                    out_sb[:, bt : bt + 1],
                    acc[:, bt, :],
                    axis=mybir.AxisListType.X,
                )
            nc.sync.dma_start(out[:], out_sb[:])

    nc.compile()
    return nc


def _get_nc():
    global _NC_CACHE
    if _NC_CACHE is None:
        _NC_CACHE = _build_bass()
    return _NC_CACHE


def kernel(x: np.ndarray, labels: np.ndarray, W: np.ndarray) -> np.ndarray:
    global LAST_RESULT
    x = np.asarray(x, dtype=np.float32)
    W = np.asarray(W, dtype=np.float32)
    labels = np.asarray(labels)

    # ---- host prep (sharding glue) ----
    norms = np.maximum(np.sqrt((x.astype(np.float64) ** 2).sum(axis=1)), 1e-12)
    xn = (x / norms[:, None].astype(np.float32)).astype(np.float32)
    # xnt[p, ko, b] = xn[b, ko*128+p] * XSCALE
    xq = (
        np.ascontiguousarray(
            (xn.T * XSCALE).reshape(KO, P, B).transpose(1, 0, 2)
        )
        .astype(ml_dtypes.float8_e4m3)
        .reshape(P, KO * B)
    )

    in_maps = []
    for i in range(N_CORES):
        shard = W[i * C_SHARD : (i + 1) * C_SHARD]
        blocks = []
        c0 = 0
        for cw, _ in CHUNK_SPEC:
            blk = (shard[c0 : c0 + cw].T * WSCALE).reshape(KO, P, cw)
            blocks.append(blk.transpose(1, 0, 2).reshape(P, KO * cw))
            c0 += cw
        wt_q = np.concatenate(blocks, axis=1).astype(ml_dtypes.float8_e4m3)
        in_maps.append({"xnt": xq, "wt": np.ascontiguousarray(wt_q)})

    # ---- device: per-core partial sum over classes of exp(s*logit) ----
    nc = _get_nc()
    res = run_bass_kernel_spmd(nc, in_maps, core_ids=list(range(N_CORES)))
    LAST_RESULT = res

    # ---- host combine (the all-reduce + tiny per-sample tail) ----
    sumexp = np.zeros(B, dtype=np.float64)
    for i in range(N_CORES):
        part = res.results[i]["out"].astype(np.float64)  # [P, B_TILES]
        sumexp += part.T.reshape(B)                      # b = bt*128 + p

    target = np.einsum(
        "bd,bd->b", xn.astype(np.float64), W[labels].astype(np.float64)
    )
    tgt = np.clip(target, -1.0 + EPS, 1.0 - EPS)
    numerator = S_SCALE * np.cos(np.arccos(tgt) + MARGIN)
    excl = sumexp - np.exp(S_SCALE * tgt)
    L = numerator - np.log(np.exp(numerator) + excl)
    return np.array(-L.mean(), dtype=np.float32)


# revision 11
# speedup vs baseline: 1.0222x; 1.0222x over previous
"""ArcFace (AngularPenaltySMLoss) distributed Trainium2 kernel, v6.

Strategy (tensor-parallel over classes, per the sharding hint):
  - Shard W's C=100000 rows over 8 cores (12500 each).
  - Host: normalize x; pre-scale and cast x, W to fp8e4m3; lay both out
    chunk-contiguously so every DMA is 128 straight partition lines.
    All input DMAs ride ONE queue in need-order (x quarter tiles and
    the small first W chunk first) so nothing is starved.
  - Device (SPMD, no collectives): per (chunk, b-tile) unit, fp8
    DoubleRow matmuls fill a [128, w] PSUM tile (si-outer / k-inner so
    column slices complete progressively). Each tile's columns are then
    consumed split across two engines, ratio ~61/39 so both hide under
    the PE stream:
      * cols [0:ca]  -> ACT: exp(2*raw) + accum_out (free-dim sum into
        the ACT accumulator tile; the exp value output goes to a dead
        SBUF dump tile so PSUM sees only reads).
      * cols [ca:w]  -> DVE: Schraudolph bit-trick exp — tensor_scalar
        affine fp32->int16 (bits of bf16 exp), then one
        scalar_tensor_tensor fold-add over the bitcast-bf16 halves with
        accum_out into the DVE accumulator tile.
    ACT and DVE accumulate into SEPARATE tiles — a shared tile would
    serialize the two engines through Tile's write-order tracking.
  - Final per-bt reduce of each accumulator + [128, 16] DMA out; host
    adds the two halves.
  - Host: sum partials over cores, compute the tiny per-sample target /
    arccos / log path in f64, return the scalar loss.
"""

import sys

if "/opt/trn_rl_repo" not in sys.path:
    sys.path.insert(0, "/opt/trn_rl_repo")

import ml_dtypes
import numpy as np

import concourse.bass as bass
import concourse.mybir as mybir
from concourse import bacc
from concourse.bass_utils import run_bass_kernel_spmd
from concourse.tile import TileContext

B, C, D = 1024, 100000, 512
S_SCALE, MARGIN, EPS = 64.0, 0.5, 1e-7
N_CORES = 8
C_SHARD = C // N_CORES          # 12500
P = 128
KO = D // P                     # 4 k-chunks of 128
B_TILES = B // P                # 8
HB = B // 2                     # x tile half-batch (512)
MM_N = 512                      # one matmul output <= one PSUM bank
N_WARM = 4                      # PE warm-up matmuls (bridge DMA fill + HAM)

WSCALE, XSCALE = 8.0, 4.0       # fp8 pre-scales (folded out via ACT_SCALE)
ACT_SCALE = S_SCALE / (WSCALE * XSCALE)   # 2.0

# Schraudolph bf16 exp bits: i16 = rint(A * raw + Badd); bitcast bf16.
# A = ACT_SCALE * 2^7/ln2; Badd = 127*2^7 - C with C calibrated to zero
# the mean relative error for s*logit ~ N(0, 1.28).
SCH_A = ACT_SCALE * 184.66496580927726
SCH_B = 16256.0 - 7.4

# (width, act_cols): per-chunk split of columns between ACT-exp and
# DVE-Schraudolph, balancing measured instruction costs:
#   ACT = 0.833*ca + ~580ns   DVE = 1.45*cs + ~440ns   PE fill = 0.84*w
CHUNK_SPEC = [
    (256, 156),
    (1748, 1052),
    (2048, 1244),
    (2048, 1244),
    (2048, 1244),
    (2048, 1244),
    (2048, 1244),
    (256, 156),
]
assert sum(w for w, _ in CHUNK_SPEC) == C_SHARD
N_CHUNKS = len(CHUNK_SPEC)

LAST_RESULT = None
_NC_CACHE = None


def _build_bass():
    nc = bacc.Bacc("TRN2")
    xnt = nc.declare_dram_parameter("xnt", [P, KO * B], mybir.dt.float8e4, isOutput=False)
    wt = nc.declare_dram_parameter("wt", [P, KO * C_SHARD], mybir.dt.float8e4, isOutput=False)
    out = nc.declare_dram_parameter("out", [P, 2 * B_TILES], mybir.dt.float32, isOutput=True)

    fp8 = mybir.dt.float8e4
    f32 = mybir.dt.float32
    bf16 = mybir.dt.bfloat16
    i16 = mybir.dt.int16
    DR = mybir.MatmulPerfMode.DoubleRow
    EXP = mybir.ActivationFunctionType.Exp

    with TileContext(nc) as tc:
        with (
            tc.tile_pool(name="xp", bufs=1) as xp,
            tc.tile_pool(name="wp", bufs=1) as wp,
            tc.tile_pool(name="ip", bufs=3) as ip,
            tc.tile_pool(name="ep", bufs=2) as ep,
            tc.tile_pool(name="fp", bufs=2) as fpool,
            tc.tile_pool(name="ac", bufs=1) as ac,
            tc.tile_pool(name="ps", bufs=2, space="PSUM") as psp,
        ):
            # x quarter tiles (k-half x batch-half) + W chunks, all on one
            # queue ordered by first use. dram xnt layout: [p, ko, b].
            xt = {}  # (khalf, bhalf) -> tile
            for kh in (0, 1):
                for bh in (0, 1):
                    xt[(kh, bh)] = xp.tile(
                        [P, 2, HB], fp8, tag=f"x{kh}{bh}", name=f"x{kh}{bh}"
                    )

            def dma_x(kh, bh):
                src = xnt.rearrange("p (ko b) -> p ko b", ko=KO)[
                    :, 2 * kh : 2 * kh + 2, bh * HB : (bh + 1) * HB
                ]
                nc.sync.dma_start(xt[(kh, bh)][:], src)

            wts = []
            c0 = 0
            for ci, (cw, _) in enumerate(CHUNK_SPEC):
                wts.append(
                    wp.tile([P, KO, cw], fp8, tag=f"wt{ci}", name=f"wt{ci}")
                )
                c0 += cw

            def dma_w(ci):
                c0 = sum(w for w, _ in CHUNK_SPEC[:ci])
                cw = CHUNK_SPEC[ci][0]
                nc.sync.dma_start(wts[ci][:], wt[:, 4 * c0 : 4 * (c0 + cw)])

            dma_x(0, 0)
            dma_w(0)
            dma_x(1, 0)
            dma_x(0, 1)
            dma_x(1, 1)
            for ci in range(1, N_CHUNKS):
                dma_w(ci)

            # ACT table warm-up: a tiny exp before any real work so the
            # ~2.7us PSEUDO_LOAD_ACT_FUNC_SET runs during the DMA fill.
            jt = xp.tile([P, 8], f32)
            nc.vector.memset(jt[:], 0.0)
            ja = xp.tile([P, 8], bf16)
            nc.scalar.activation(ja[:], jt[:], EXP)

            # PE warm-up: bridge from engine start to the first
            # data-dependent matmul so HAM un-throttles (~3.4us window).
            wsrc = xp.tile([P, MM_N], fp8, tag="warm_src")
            nc.vector.memset(wsrc[:], 1)
            for _ in range(N_WARM):
                pw = psp.tile([P, 2048], f32, tag="ps")
                nc.tensor.matmul(
                    pw[:, :MM_N], wsrc[:, :P], wsrc[:], start=True, stop=True
                )

            # separate accumulators per engine (shared tile would
            # serialize ACT and DVE through write-order tracking)
            acc_a = ac.tile([P, B_TILES, N_CHUNKS], f32)
            acc_d = ac.tile([P, B_TILES, N_CHUNKS], f32)
            out_sb = ac.tile([P, 2, B_TILES], f32)

            for ci, (cw, ca) in enumerate(CHUNK_SPEC):
                wt_t = wts[ci]
                cs = cw - ca
                h = cs // 2
                n_sub = (cw + MM_N - 1) // MM_N
                for bt in range(B_TILES):
                    bh, bo = divmod(bt, 4)
                    ps = psp.tile([P, 2048], f32, tag="ps")
                    # si-outer / k-inner: column slices complete
                    # progressively so consumers can start early.
                    for si in range(n_sub):
                        s0 = si * MM_N
                        sw = min(MM_N, cw - s0)
                        for k in (0, 1):
                            nc.tensor.matmul(
                                ps[:, s0 : s0 + sw],
                                xt[(k, bh)][:, :, bo * P : (bo + 1) * P],
                                wt_t[:, 2 * k : 2 * k + 2, s0 : s0 + sw],
                                start=(k == 0),
                                stop=(k == 1),
                                perf_mode=DR,
                            )
                    # ACT cols: exp + free-dim accumulate (value output
                    # goes to a dead SBUF dump tile).
                    exd = ep.tile([P, 1280], bf16, tag="exd")
                    nc.scalar.activation(
                        exd[:, :ca],
                        ps[:, :ca],
                        EXP,
                        scale=ACT_SCALE,
                        accum_out=acc_a[:, bt, ci : ci + 1],
                    )
                    # DVE cols: Schraudolph exp bits + fused fold+accum
                    it = ip.tile([P, 1024], i16, tag="it")
                    nc.vector.tensor_scalar(
                        it[:, :cs],
                        ps[:, ca:cw],
                        SCH_A,
                        SCH_B,
                        mybir.AluOpType.mult,
                        mybir.AluOpType.add,
                    )
                    fo = fpool.tile([P, 512], bf16, tag="fo")
                    nc.vector.scalar_tensor_tensor(
                        fo[:, :h],
                        it[:, 0:h].bitcast(bf16),
                        1.0,
                        it[:, h:cs].bitcast(bf16),
                        mybir.AluOpType.mult,
                        mybir.AluOpType.add,
                        accum_out=acc_d[:, bt, ci : ci + 1],
                    )

            for bt in range(B_TILES):
                nc.vector.reduce_sum(
                    out_sb[:, 1, bt : bt + 1],
                    acc_d[:, bt, :],
                    axis=mybir.AxisListType.X,
                )
                nc.vector.reduce_sum(
                    out_sb[:, 0, bt : bt + 1],
                    acc_a[:, bt, :],
                    axis=mybir.AxisListType.X,
                )
            nc.scalar.dma_start(out[:], out_sb[:])

    nc.compile()
    return nc


def _get_nc():
    global _NC_CACHE
    if _NC_CACHE is None:
        _NC_CACHE = _build_bass()
    return _NC_CACHE


def kernel(x: np.ndarray, labels: np.ndarray, W: np.ndarray) -> np.ndarray:
    global LAST_RESULT
    x = np.asarray(x, dtype=np.float32)
    W = np.asarray(W, dtype=np.float32)
    labels = np.asarray(labels)

    # ---- host prep (sharding glue) ----
    norms = np.maximum(np.sqrt((x.astype(np.float64) ** 2).sum(axis=1)), 1e-12)
    xn = (x / norms[:, None].astype(np.float32)).astype(np.float32)
    # xnt[p, ko, b] = xn[b, ko*128+p] * XSCALE
    xq = (
        np.ascontiguousarray(
            (xn.T * XSCALE).reshape(KO, P, B).transpose(1, 0, 2)
        )
        .astype(ml_dtypes.float8_e4m3)
        .reshape(P, KO * B)
    )

    in_maps = []
    for i in range(N_CORES):
        shard = W[i * C_SHARD : (i + 1) * C_SHARD]
        blocks = []
        c0 = 0
        for cw, _ in CHUNK_SPEC:
            blk = (shard[c0 : c0 + cw].T * WSCALE).reshape(KO, P, cw)
            blocks.append(blk.transpose(1, 0, 2).reshape(P, KO * cw))
            c0 += cw
        wt_q = np.concatenate(blocks, axis=1).astype(ml_dtypes.float8_e4m3)
        in_maps.append({"xnt": xq, "wt": np.ascontiguousarray(wt_q)})

    # ---- device: per-core partial sum over classes of exp(s*logit) ----
    nc = _get_nc()
    res = run_bass_kernel_spmd(nc, in_maps, core_ids=list(range(N_CORES)))
    LAST_RESULT = res

    # ---- host combine (the all-reduce + tiny per-sample tail) ----
    sumexp = np.zeros(B, dtype=np.float64)
    for i in range(N_CORES):
        part = res.results[i]["out"].astype(np.float64)  # [P, 2, B_TILES]
        part = part.reshape(P, 2, B_TILES).sum(axis=1)   # [P, B_TILES]
        sumexp += part.T.reshape(B)                      # b = bt*128 + p

    target = np.einsum(
        "bd,bd->b", xn.astype(np.float64), W[labels].astype(np.float64)
    )
    tgt = np.clip(target, -1.0 + EPS, 1.0 - EPS)
    numerator = S_SCALE * np.cos(np.arccos(tgt) + MARGIN)
    excl = sumexp - np.exp(S_SCALE * tgt)
    L = numerator - np.log(np.exp(numerator) + excl)
    return np.array(-L.mean(), dtype=np.float32)


# revision 13
# speedup vs baseline: 1.0276x; 1.0053x over previous
"""ArcFace (AngularPenaltySMLoss) distributed Trainium2 kernel, v6.

Strategy (tensor-parallel over classes, per the sharding hint):
  - Shard W's C=100000 rows over 8 cores (12500 each).
  - Host: normalize x; pre-scale and cast x, W to fp8e4m3; lay both out
    chunk-contiguously so every DMA is 128 straight partition lines.
    All input DMAs ride ONE queue in need-order (x quarter tiles and
    the small first W chunk first) so nothing is starved.
  - Device (SPMD, no collectives): per (chunk, b-tile) unit, fp8
    DoubleRow matmuls fill a [128, w] PSUM tile (si-outer / k-inner so
    column slices complete progressively). Each tile's columns are then
    consumed split across two engines, ratio ~61/39 so both hide under
    the PE stream:
      * cols [0:ca]  -> ACT: exp(2*raw) + accum_out (free-dim sum into
        the ACT accumulator tile; the exp value output goes to a dead
        SBUF dump tile so PSUM sees only reads).
      * cols [ca:w]  -> DVE: Schraudolph bit-trick exp — tensor_scalar
        affine fp32->int16 (bits of bf16 exp), then one
        scalar_tensor_tensor fold-add over the bitcast-bf16 halves with
        accum_out into the DVE accumulator tile.
    ACT and DVE accumulate into SEPARATE tiles — a shared tile would
    serialize the two engines through Tile's write-order tracking.
  - Final per-bt reduce of each accumulator + [128, 16] DMA out; host
    adds the two halves.
  - Host: sum partials over cores, compute the tiny per-sample target /
    arccos / log path in f64, return the scalar loss.
"""

import sys

if "/opt/trn_rl_repo" not in sys.path:
    sys.path.insert(0, "/opt/trn_rl_repo")

import ml_dtypes
import numpy as np

import concourse.bass as bass
import concourse.mybir as mybir
from concourse import bacc
from concourse.bass_utils import run_bass_kernel_spmd
from concourse.tile import TileContext

B, C, D = 1024, 100000, 512
S_SCALE, MARGIN, EPS = 64.0, 0.5, 1e-7
N_CORES = 8
C_SHARD = C // N_CORES          # 12500
P = 128
KO = D // P                     # 4 k-chunks of 128
B_TILES = B // P                # 8
HB = B // 2                     # x tile half-batch (512)
MM_N = 512                      # one matmul output <= one PSUM bank
N_WARM = 4                      # PE warm-up matmuls (bridge DMA fill + HAM)

WSCALE, XSCALE = 8.0, 4.0       # fp8 pre-scales (folded out via ACT_SCALE)
ACT_SCALE = S_SCALE / (WSCALE * XSCALE)   # 2.0

# Schraudolph bf16 exp bits: i16 = rint(A * raw + Badd); bitcast bf16.
# A = ACT_SCALE * 2^7/ln2; Badd = 127*2^7 - C with C calibrated to zero
# the mean relative error for s*logit ~ N(0, 1.28).
SCH_A = ACT_SCALE * 184.66496580927726
SCH_B = 16256.0 - 7.4

# (width, act_cols): per-chunk split of columns between ACT-exp and
# DVE-Schraudolph, balancing measured instruction costs:
#   ACT = 0.833*ca + ~580ns   DVE = 1.45*cs + ~440ns   PE fill = 0.84*w
CHUNK_SPEC = [
    (256, 156),
    (1748, 1052),
    (2048, 1244),
    (2048, 1244),
    (2048, 1244),
    (2048, 1244),
    (2048, 1244),
    (256, 156),
]
assert sum(w for w, _ in CHUNK_SPEC) == C_SHARD
N_CHUNKS = len(CHUNK_SPEC)

LAST_RESULT = None
_NC_CACHE = None


def _build_bass():
    nc = bacc.Bacc("TRN2")
    xnt = nc.declare_dram_parameter("xnt", [P, KO * B], mybir.dt.float8e4, isOutput=False)
    wt = nc.declare_dram_parameter("wt", [P, KO * C_SHARD], mybir.dt.float8e4, isOutput=False)
    out = nc.declare_dram_parameter("out", [P, 2 * B_TILES], mybir.dt.float32, isOutput=True)

    fp8 = mybir.dt.float8e4
    f32 = mybir.dt.float32
    bf16 = mybir.dt.bfloat16
    i16 = mybir.dt.int16
    DR = mybir.MatmulPerfMode.DoubleRow
    EXP = mybir.ActivationFunctionType.Exp

    with TileContext(nc) as tc:
        with (
            tc.tile_pool(name="xp", bufs=1) as xp,
            tc.tile_pool(name="wp", bufs=1) as wp,
            tc.tile_pool(name="ip", bufs=3) as ip,
            tc.tile_pool(name="ep", bufs=2) as ep,
            tc.tile_pool(name="fp", bufs=2) as fpool,
            tc.tile_pool(name="ac", bufs=1) as ac,
            # two alternating PSUM pools: Tile chains a pool slot's readers
            # (DVE ts waits the ACT accum-read), so a single 2-buf pool
            # couples the engines; alternating pools gives each chain two
            # unit-periods of slack.
            tc.tile_pool(name="psA", bufs=1, space="PSUM") as psp_a,
            tc.tile_pool(name="psB", bufs=1, space="PSUM") as psp_b,
        ):
            # x quarter tiles (k-half x batch-half) + W chunks, all on one
            # queue ordered by first use. dram xnt layout: [p, ko, b].
            xt = {}  # (khalf, bhalf) -> tile
            for kh in (0, 1):
                for bh in (0, 1):
                    xt[(kh, bh)] = xp.tile(
                        [P, 2, HB], fp8, tag=f"x{kh}{bh}", name=f"x{kh}{bh}"
                    )

            def dma_x(kh, bh):
                src = xnt.rearrange("p (ko b) -> p ko b", ko=KO)[
                    :, 2 * kh : 2 * kh + 2, bh * HB : (bh + 1) * HB
                ]
                nc.sync.dma_start(xt[(kh, bh)][:], src)

            wts = []
            c0 = 0
            for ci, (cw, _) in enumerate(CHUNK_SPEC):
                wts.append(
                    wp.tile([P, KO, cw], fp8, tag=f"wt{ci}", name=f"wt{ci}")
                )
                c0 += cw

            def dma_w(ci):
                c0 = sum(w for w, _ in CHUNK_SPEC[:ci])
                cw = CHUNK_SPEC[ci][0]
                nc.sync.dma_start(wts[ci][:], wt[:, 4 * c0 : 4 * (c0 + cw)])

            dma_x(0, 0)
            dma_w(0)
            dma_x(1, 0)
            dma_x(0, 1)
            dma_x(1, 1)
            for ci in range(1, N_CHUNKS):
                dma_w(ci)

            # ACT table warm-up: a tiny exp before any real work so the
            # ~2.7us PSEUDO_LOAD_ACT_FUNC_SET runs during the DMA fill.
            jt = xp.tile([P, 8], f32)
            nc.vector.memset(jt[:], 0.0)
            ja = xp.tile([P, 8], bf16)
            nc.scalar.activation(ja[:], jt[:], EXP)

            # PE warm-up: bridge from engine start to the first
            # data-dependent matmul so HAM un-throttles (~3.4us window).
            wsrc = xp.tile([P, MM_N], fp8, tag="warm_src")
            nc.vector.memset(wsrc[:], 1)
            for wi in range(N_WARM):
                pw = (psp_a if wi % 2 == 0 else psp_b).tile(
                    [P, 2048], f32, tag="ps"
                )
                nc.tensor.matmul(
                    pw[:, :MM_N], wsrc[:, :P], wsrc[:], start=True, stop=True
                )

            # separate accumulators per engine (shared tile would
            # serialize ACT and DVE through write-order tracking)
            acc_a = ac.tile([P, B_TILES, N_CHUNKS], f32)
            acc_d = ac.tile([P, B_TILES, N_CHUNKS], f32)
            out_sb = ac.tile([P, 2, B_TILES], f32)

            for ci, (cw, ca) in enumerate(CHUNK_SPEC):
                wt_t = wts[ci]
                cs = cw - ca
                h = cs // 2
                n_sub = (cw + MM_N - 1) // MM_N
                for bt in range(B_TILES):
                    bh, bo = divmod(bt, 4)
                    ui = ci * B_TILES + bt
                    ps = (psp_a if ui % 2 == 0 else psp_b).tile(
                        [P, 2048], f32, tag="ps"
                    )
                    # si-outer / k-inner: column slices complete
                    # progressively so consumers can start early.
                    for si in range(n_sub):
                        s0 = si * MM_N
                        sw = min(MM_N, cw - s0)
                        for k in (0, 1):
                            nc.tensor.matmul(
                                ps[:, s0 : s0 + sw],
                                xt[(k, bh)][:, :, bo * P : (bo + 1) * P],
                                wt_t[:, 2 * k : 2 * k + 2, s0 : s0 + sw],
                                start=(k == 0),
                                stop=(k == 1),
                                perf_mode=DR,
                            )
                    # ACT cols: exp + free-dim accumulate (value output
                    # goes to a dead SBUF dump tile).
                    exd = ep.tile([P, 1280], bf16, tag="exd")
                    nc.scalar.activation(
                        exd[:, :ca],
                        ps[:, :ca],
                        EXP,
                        scale=ACT_SCALE,
                        accum_out=acc_a[:, bt, ci : ci + 1],
                    )
                    # DVE cols: Schraudolph exp bits + fused fold+accum
                    it = ip.tile([P, 1024], i16, tag="it")
                    nc.vector.tensor_scalar(
                        it[:, :cs],
                        ps[:, ca:cw],
                        SCH_A,
                        SCH_B,
                        mybir.AluOpType.mult,
                        mybir.AluOpType.add,
                    )
                    fo = fpool.tile([P, 512], bf16, tag="fo")
                    nc.vector.scalar_tensor_tensor(
                        fo[:, :h],
                        it[:, 0:h].bitcast(bf16),
                        1.0,
                        it[:, h:cs].bitcast(bf16),
                        mybir.AluOpType.mult,
                        mybir.AluOpType.add,
                        accum_out=acc_d[:, bt, ci : ci + 1],
                    )

            for bt in range(B_TILES):
                nc.vector.reduce_sum(
                    out_sb[:, 1, bt : bt + 1],
                    acc_d[:, bt, :],
                    axis=mybir.AxisListType.X,
                )
                nc.vector.reduce_sum(
                    out_sb[:, 0, bt : bt + 1],
                    acc_a[:, bt, :],
                    axis=mybir.AxisListType.X,
                )
            nc.scalar.dma_start(out[:], out_sb[:])

    nc.compile()
    return nc


def _get_nc():
    global _NC_CACHE
    if _NC_CACHE is None:
        _NC_CACHE = _build_bass()
    return _NC_CACHE


def kernel(x: np.ndarray, labels: np.ndarray, W: np.ndarray) -> np.ndarray:
    global LAST_RESULT
    x = np.asarray(x, dtype=np.float32)
    W = np.asarray(W, dtype=np.float32)
    labels = np.asarray(labels)

    # ---- host prep (sharding glue) ----
    norms = np.maximum(np.sqrt((x.astype(np.float64) ** 2).sum(axis=1)), 1e-12)
    xn = (x / norms[:, None].astype(np.float32)).astype(np.float32)
    # xnt[p, ko, b] = xn[b, ko*128+p] * XSCALE
    xq = (
        np.ascontiguousarray(
            (xn.T * XSCALE).reshape(KO, P, B).transpose(1, 0, 2)
        )
        .astype(ml_dtypes.float8_e4m3)
        .reshape(P, KO * B)
    )

    in_maps = []
    for i in range(N_CORES):
        shard = W[i * C_SHARD : (i + 1) * C_SHARD]
        blocks = []
        c0 = 0
        for cw, _ in CHUNK_SPEC:
            blk = (shard[c0 : c0 + cw].T * WSCALE).reshape(KO, P, cw)
            blocks.append(blk.transpose(1, 0, 2).reshape(P, KO * cw))
            c0 += cw
        wt_q = np.concatenate(blocks, axis=1).astype(ml_dtypes.float8_e4m3)
        in_maps.append({"xnt": xq, "wt": np.ascontiguousarray(wt_q)})

    # ---- device: per-core partial sum over classes of exp(s*logit) ----
    nc = _get_nc()
    res = run_bass_kernel_spmd(nc, in_maps, core_ids=list(range(N_CORES)))
    LAST_RESULT = res

    # ---- host combine (the all-reduce + tiny per-sample tail) ----
    sumexp = np.zeros(B, dtype=np.float64)
    for i in range(N_CORES):
        part = res.results[i]["out"].astype(np.float64)  # [P, 2, B_TILES]
        part = part.reshape(P, 2, B_TILES).sum(axis=1)   # [P, B_TILES]
        sumexp += part.T.reshape(B)                      # b = bt*128 + p

    target = np.einsum(
        "bd,bd->b", xn.astype(np.float64), W[labels].astype(np.float64)
    )
    tgt = np.clip(target, -1.0 + EPS, 1.0 - EPS)
    numerator = S_SCALE * np.cos(np.arccos(tgt) + MARGIN)
    excl = sumexp - np.exp(S_SCALE * tgt)
    L = numerator - np.log(np.exp(numerator) + excl)
    return np.array(-L.mean(), dtype=np.float32)


# revision 14
# speedup vs baseline: 1.0885x; 1.0593x over previous
"""ArcFace (AngularPenaltySMLoss) distributed Trainium2 kernel, v6.

Strategy (tensor-parallel over classes, per the sharding hint):
  - Shard W's C=100000 rows over 8 cores (12500 each).
  - Host: normalize x; pre-scale and cast x, W to fp8e4m3; lay both out
    chunk-contiguously so every DMA is 128 straight partition lines.
    All input DMAs ride ONE queue in need-order (x quarter tiles and
    the small first W chunk first) so nothing is starved.
  - Device (SPMD, no collectives): per (chunk, b-tile) unit, fp8
    DoubleRow matmuls fill a [128, w] PSUM tile (si-outer / k-inner so
    column slices complete progressively). Each tile's columns are then
    consumed split across two engines, ratio ~61/39 so both hide under
    the PE stream:
      * cols [0:ca]  -> ACT: exp(2*raw) + accum_out (free-dim sum into
        the ACT accumulator tile; the exp value output goes to a dead
        SBUF dump tile so PSUM sees only reads).
      * cols [ca:w]  -> DVE: Schraudolph bit-trick exp — tensor_scalar
        affine fp32->int16 (bits of bf16 exp), then one
        scalar_tensor_tensor fold-add over the bitcast-bf16 halves with
        accum_out into the DVE accumulator tile.
    ACT and DVE accumulate into SEPARATE tiles — a shared tile would
    serialize the two engines through Tile's write-order tracking.
  - Final per-bt reduce of each accumulator + [128, 16] DMA out; host
    adds the two halves.
  - Host: sum partials over cores, compute the tiny per-sample target /
    arccos / log path in f64, return the scalar loss.
"""

import sys

if "/opt/trn_rl_repo" not in sys.path:
    sys.path.insert(0, "/opt/trn_rl_repo")

import ml_dtypes
import numpy as np

import concourse.bass as bass
import concourse.mybir as mybir
from concourse import bacc
from concourse.bass_utils import run_bass_kernel_spmd
from concourse.tile import TileContext

B, C, D = 1024, 100000, 512
S_SCALE, MARGIN, EPS = 64.0, 0.5, 1e-7
N_CORES = 8
C_SHARD = C // N_CORES          # 12500
P = 128
KO = D // P                     # 4 k-chunks of 128
B_TILES = B // P                # 8
HB = B // 2                     # x tile half-batch (512)
MM_N = 512                      # one matmul output <= one PSUM bank
N_WARM = 4                      # PE warm-up matmuls (bridge DMA fill + HAM)

WSCALE, XSCALE = 8.0, 4.0       # fp8 pre-scales (folded out via ACT_SCALE)
ACT_SCALE = S_SCALE / (WSCALE * XSCALE)   # 2.0

# Schraudolph bf16 exp bits: i16 = rint(A * raw + Badd); bitcast bf16.
# A = ACT_SCALE * 2^7/ln2; Badd = 127*2^7 - C with C calibrated to zero
# the mean relative error for s*logit ~ N(0, 1.28).
SCH_A = ACT_SCALE * 184.66496580927726
SCH_B = 16256.0 - 7.4

# (width, act_cols): per-chunk split of columns between ACT-exp and
# DVE-Schraudolph, balancing measured instruction costs:
#   ACT = 0.833*ca + ~580ns   DVE = 1.45*cs + ~440ns   PE fill = 0.84*w
# (width, dve_cols): DVE consumes cols [0:cs] (ready after the first two
# 512-col sub-fills, so its chain link starts early); ACT consumes
# [cs:w]. Emitted DVE-first so Tile's PSUM access chain is
# ts -> exp and fits two fill periods.
CHUNK_SPEC = [
    (304, 104),
    (1956, 760),
    (2048, 768),
    (2048, 768),
    (2048, 768),
    (2048, 768),
    (2048, 768),
]
assert sum(w for w, _ in CHUNK_SPEC) == C_SHARD
N_CHUNKS = len(CHUNK_SPEC)

LAST_RESULT = None
_NC_CACHE = None


def _build_bass():
    nc = bacc.Bacc("TRN2")
    xnt = nc.declare_dram_parameter("xnt", [P, KO * B], mybir.dt.float8e4, isOutput=False)
    wt = nc.declare_dram_parameter("wt", [P, KO * C_SHARD], mybir.dt.float8e4, isOutput=False)
    out = nc.declare_dram_parameter("out", [P, 2 * B_TILES], mybir.dt.float32, isOutput=True)

    fp8 = mybir.dt.float8e4
    f32 = mybir.dt.float32
    bf16 = mybir.dt.bfloat16
    i16 = mybir.dt.int16
    DR = mybir.MatmulPerfMode.DoubleRow
    EXP = mybir.ActivationFunctionType.Exp

    with TileContext(nc) as tc:
        with (
            tc.tile_pool(name="xp", bufs=1) as xp,
            tc.tile_pool(name="wp", bufs=1) as wp,
            tc.tile_pool(name="ip", bufs=3) as ip,
            tc.tile_pool(name="ep", bufs=2) as ep,
            tc.tile_pool(name="fp", bufs=2) as fpool,
            tc.tile_pool(name="ac", bufs=1) as ac,
            # two alternating PSUM pools: Tile chains a pool slot's readers
            # (DVE ts waits the ACT accum-read), so a single 2-buf pool
            # couples the engines; alternating pools gives each chain two
            # unit-periods of slack.
            tc.tile_pool(name="psA", bufs=1, space="PSUM") as psp_a,
            tc.tile_pool(name="psB", bufs=1, space="PSUM") as psp_b,
        ):
            # x quarter tiles (k-half x batch-half) + W chunks, all on one
            # queue ordered by first use. dram xnt layout: [p, ko, b].
            xt = {}  # (khalf, bhalf) -> tile
            for kh in (0, 1):
                for bh in (0, 1):
                    xt[(kh, bh)] = xp.tile(
                        [P, 2, HB], fp8, tag=f"x{kh}{bh}", name=f"x{kh}{bh}"
                    )

            def dma_x(kh, bh):
                src = xnt.rearrange("p (ko b) -> p ko b", ko=KO)[
                    :, 2 * kh : 2 * kh + 2, bh * HB : (bh + 1) * HB
                ]
                nc.sync.dma_start(xt[(kh, bh)][:], src)

            wts = []
            c0 = 0
            for ci, (cw, _) in enumerate(CHUNK_SPEC):
                wts.append(
                    wp.tile([P, KO, cw], fp8, tag=f"wt{ci}", name=f"wt{ci}")
                )
                c0 += cw

            def dma_w(ci):
                c0 = sum(w for w, _ in CHUNK_SPEC[:ci])
                cw = CHUNK_SPEC[ci][0]
                nc.sync.dma_start(wts[ci][:], wt[:, 4 * c0 : 4 * (c0 + cw)])

            dma_x(0, 0)
            dma_w(0)
            dma_x(1, 0)
            dma_x(0, 1)
            dma_x(1, 1)
            for ci in range(1, N_CHUNKS):
                dma_w(ci)

            # ACT table warm-up: a tiny exp before any real work so the
            # ~2.7us PSEUDO_LOAD_ACT_FUNC_SET runs during the DMA fill.
            jt = xp.tile([P, 8], f32)
            nc.vector.memset(jt[:], 0.0)
            ja = xp.tile([P, 8], bf16)
            nc.scalar.activation(ja[:], jt[:], EXP)

            # PE warm-up: bridge from engine start to the first
            # data-dependent matmul so HAM un-throttles (~3.4us window).
            wsrc = xp.tile([P, MM_N], fp8, tag="warm_src")
            nc.vector.memset(wsrc[:], 1)
            for wi in range(N_WARM):
                pw = (psp_a if wi % 2 == 0 else psp_b).tile(
                    [P, 2048], f32, tag="ps"
                )
                nc.tensor.matmul(
                    pw[:, :MM_N], wsrc[:, :P], wsrc[:], start=True, stop=True
                )

            # separate accumulators per engine (shared tile would
            # serialize ACT and DVE through write-order tracking)
            acc_a = ac.tile([P, B_TILES, N_CHUNKS], f32)
            acc_d = ac.tile([P, B_TILES, N_CHUNKS], f32)
            out_sb = ac.tile([P, 2, B_TILES], f32)

            for ci, (cw, cs) in enumerate(CHUNK_SPEC):
                wt_t = wts[ci]
                ca = cw - cs
                h = cs // 2
                n_sub = (cw + MM_N - 1) // MM_N
                for bt in range(B_TILES):
                    bh, bo = divmod(bt, 4)
                    ui = ci * B_TILES + bt
                    ps = (psp_a if ui % 2 == 0 else psp_b).tile(
                        [P, 2048], f32, tag="ps"
                    )
                    # si-outer / k-inner: column slices complete
                    # progressively so consumers can start early.
                    for si in range(n_sub):
                        s0 = si * MM_N
                        sw = min(MM_N, cw - s0)
                        for k in (0, 1):
                            nc.tensor.matmul(
                                ps[:, s0 : s0 + sw],
                                xt[(k, bh)][:, :, bo * P : (bo + 1) * P],
                                wt_t[:, 2 * k : 2 * k + 2, s0 : s0 + sw],
                                start=(k == 0),
                                stop=(k == 1),
                                perf_mode=DR,
                            )
                    # DVE cols first (emission order = Tile's PSUM access
                    # chain order): Schraudolph exp bits + fused fold+accum
                    it = ip.tile([P, 1024], i16, tag="it")
                    nc.vector.tensor_scalar(
                        it[:, :cs],
                        ps[:, 0:cs],
                        SCH_A,
                        SCH_B,
                        mybir.AluOpType.mult,
                        mybir.AluOpType.add,
                    )
                    fo = fpool.tile([P, 512], bf16, tag="fo")
                    nc.vector.scalar_tensor_tensor(
                        fo[:, :h],
                        it[:, 0:h].bitcast(bf16),
                        1.0,
                        it[:, h:cs].bitcast(bf16),
                        mybir.AluOpType.mult,
                        mybir.AluOpType.add,
                        accum_out=acc_d[:, bt, ci : ci + 1],
                    )
                    # ACT cols: exp + free-dim accumulate (value output
                    # goes to a dead SBUF dump tile).
                    exd = ep.tile([P, 1408], bf16, tag="exd")
                    nc.scalar.activation(
                        exd[:, :ca],
                        ps[:, cs:cw],
                        EXP,
                        scale=ACT_SCALE,
                        accum_out=acc_a[:, bt, ci : ci + 1],
                    )

            for bt in range(B_TILES):
                nc.vector.reduce_sum(
                    out_sb[:, 1, bt : bt + 1],
                    acc_d[:, bt, :],
                    axis=mybir.AxisListType.X,
                )
                nc.vector.reduce_sum(
                    out_sb[:, 0, bt : bt + 1],
                    acc_a[:, bt, :],
                    axis=mybir.AxisListType.X,
                )
            nc.scalar.dma_start(out[:], out_sb[:])

    nc.compile()
    return nc


def _get_nc():
    global _NC_CACHE
    if _NC_CACHE is None:
        _NC_CACHE = _build_bass()
    return _NC_CACHE


def kernel(x: np.ndarray, labels: np.ndarray, W: np.ndarray) -> np.ndarray:
    global LAST_RESULT
    x = np.asarray(x, dtype=np.float32)
    W = np.asarray(W, dtype=np.float32)
    labels = np.asarray(labels)

    # ---- host prep (sharding glue) ----
    norms = np.maximum(np.sqrt((x.astype(np.float64) ** 2).sum(axis=1)), 1e-12)
    xn = (x / norms[:, None].astype(np.float32)).astype(np.float32)
    # xnt[p, ko, b] = xn[b, ko*128+p] * XSCALE
    xq = (
        np.ascontiguousarray(
            (xn.T * XSCALE).reshape(KO, P, B).transpose(1, 0, 2)
        )
        .astype(ml_dtypes.float8_e4m3)
        .reshape(P, KO * B)
    )

    in_maps = []
    for i in range(N_CORES):
        shard = W[i * C_SHARD : (i + 1) * C_SHARD]
        blocks = []
        c0 = 0
        for cw, _ in CHUNK_SPEC:
            blk = (shard[c0 : c0 + cw].T * WSCALE).reshape(KO, P, cw)
            blocks.append(blk.transpose(1, 0, 2).reshape(P, KO * cw))
            c0 += cw
        wt_q = np.concatenate(blocks, axis=1).astype(ml_dtypes.float8_e4m3)
        in_maps.append({"xnt": xq, "wt": np.ascontiguousarray(wt_q)})

    # ---- device: per-core partial sum over classes of exp(s*logit) ----
    nc = _get_nc()
    res = run_bass_kernel_spmd(nc, in_maps, core_ids=list(range(N_CORES)))
    LAST_RESULT = res

    # ---- host combine (the all-reduce + tiny per-sample tail) ----
    sumexp = np.zeros(B, dtype=np.float64)
    for i in range(N_CORES):
        part = res.results[i]["out"].astype(np.float64)  # [P, 2, B_TILES]
        part = part.reshape(P, 2, B_TILES).sum(axis=1)   # [P, B_TILES]
        sumexp += part.T.reshape(B)                      # b = bt*128 + p

    target = np.einsum(
        "bd,bd->b", xn.astype(np.float64), W[labels].astype(np.float64)
    )
    tgt = np.clip(target, -1.0 + EPS, 1.0 - EPS)
    numerator = S_SCALE * np.cos(np.arccos(tgt) + MARGIN)
    excl = sumexp - np.exp(S_SCALE * tgt)
    L = numerator - np.log(np.exp(numerator) + excl)
    return np.array(-L.mean(), dtype=np.float32)


# revision 15
# speedup vs baseline: 1.2831x; 1.1787x over previous
"""ArcFace (AngularPenaltySMLoss) distributed Trainium2 kernel, v8.

Strategy (tensor-parallel over classes, per the sharding hint):
  - Shard W's C=100000 rows over 8 cores (12500 each).
  - Host: normalize x; pre-scale and cast x, W to fp8e4m3; lay both out
    chunk-contiguously so every DMA is 128 straight partition lines.
    All input DMAs ride ONE queue in need-order (x quarter tiles and
    the small first W chunk first) so nothing is starved.
  - Device (SPMD, no collectives): per (chunk, b-tile) unit, fp8
    DoubleRow matmuls fill TWO bank-aligned PSUM tiles: psa (cols
    0:1024) and psd (cols 1024:w). Tile/PSUM semantics serialize all
    consumers of one PSUM tile and make them wait for the whole fill,
    so each tile gets exactly ONE consumer:
      * psa -> ACT: exp(2*raw) + accum_out (free-dim sum straight into
        an accumulator slot; the exp value output goes to a dead SBUF
        dump tile).
      * psd -> DVE: Schraudolph bit-trick exp — tensor_scalar affine
        fp32->int16 (bits of bf16 exp), then one scalar_tensor_tensor
        fold-add over the bitcast-bf16 halves with accum_out. Every
        7th unit ACT takes psd instead (exp+accum) to balance engine
        load (~95% each, under the PE fill rate).
    ACT and DVE accumulate into SEPARATE tiles — a shared tile would
    serialize the two engines through Tile's write-order tracking.
  - Final per-bt reduce of each accumulator + [128, 24] DMA out; host
    adds the halves.
  - Host: sum partials over cores, compute the tiny per-sample target /
    arccos / log path in f64, return the scalar loss.
"""

import sys

if "/opt/trn_rl_repo" not in sys.path:
    sys.path.insert(0, "/opt/trn_rl_repo")

import ml_dtypes
import numpy as np

import concourse.bass as bass
import concourse.mybir as mybir
from concourse import bacc
from concourse.bass_utils import run_bass_kernel_spmd
from concourse.tile import TileContext

B, C, D = 1024, 100000, 512
S_SCALE, MARGIN, EPS = 64.0, 0.5, 1e-7
N_CORES = 8
C_SHARD = C // N_CORES          # 12500
P = 128
KO = D // P                     # 4 k-chunks of 128
B_TILES = B // P                # 8
HB = B // 2                     # x tile half-batch (512)
MM_N = 512                      # one matmul output <= one PSUM bank
HALF = 1024                     # psa width (2 PSUM banks)
N_WARM = 4                      # PE warm-up matmuls (bridge DMA fill + HAM)

WSCALE, XSCALE = 8.0, 4.0       # fp8 pre-scales (folded out via ACT_SCALE)
ACT_SCALE = S_SCALE / (WSCALE * XSCALE)   # 2.0

# Schraudolph bf16 exp bits: i16 = rint(A * raw + Badd); bitcast bf16.
SCH_A = ACT_SCALE * 184.66496580927726
SCH_B = 16256.0 - 7.4

CHUNKS = [212, 2048, 2048, 2048, 2048, 2048, 2048]
assert sum(CHUNKS) == C_SHARD
N_CHUNKS = len(CHUNKS)
ACT_PSD_EVERY = 7               # every 7th unit ACT consumes psd too

LAST_RESULT = None
_NC_CACHE = None


def _build_bass():
    nc = bacc.Bacc("TRN2")
    xnt = nc.declare_dram_parameter("xnt", [P, KO * B], mybir.dt.float8e4, isOutput=False)
    wt = nc.declare_dram_parameter("wt", [P, KO * C_SHARD], mybir.dt.float8e4, isOutput=False)
    out = nc.declare_dram_parameter("out", [P, 3 * B_TILES], mybir.dt.float32, isOutput=True)

    fp8 = mybir.dt.float8e4
    f32 = mybir.dt.float32
    bf16 = mybir.dt.bfloat16
    i16 = mybir.dt.int16
    DR = mybir.MatmulPerfMode.DoubleRow
    EXP = mybir.ActivationFunctionType.Exp

    with TileContext(nc) as tc:
        with (
            tc.tile_pool(name="xp", bufs=1) as xp,
            tc.tile_pool(name="wp", bufs=1) as wp,
            tc.tile_pool(name="ip", bufs=3) as ip,
            tc.tile_pool(name="ep", bufs=3) as ep,
            tc.tile_pool(name="fp", bufs=2) as fpool,
            tc.tile_pool(name="ac", bufs=1) as ac,
            # one PSUM pool per consumer engine; single reader per tile
            tc.tile_pool(name="psa", bufs=2, space="PSUM") as psa_p,
            tc.tile_pool(name="psd", bufs=2, space="PSUM") as psd_p,
        ):
            # x quarter tiles (k-half x batch-half) + W chunks, all on one
            # queue ordered by first use. dram xnt layout: [p, ko, b].
            xt = {}
            for kh in (0, 1):
                for bh in (0, 1):
                    xt[(kh, bh)] = xp.tile(
                        [P, 2, HB], fp8, tag=f"x{kh}{bh}", name=f"x{kh}{bh}"
                    )

            def dma_x(kh, bh):
                src = xnt.rearrange("p (ko b) -> p ko b", ko=KO)[
                    :, 2 * kh : 2 * kh + 2, bh * HB : (bh + 1) * HB
                ]
                nc.sync.dma_start(xt[(kh, bh)][:], src)

            wts = []
            for ci, cw in enumerate(CHUNKS):
                wts.append(
                    wp.tile([P, KO, cw], fp8, tag=f"wt{ci}", name=f"wt{ci}")
                )

            def dma_w(ci):
                c0 = sum(CHUNKS[:ci])
                cw = CHUNKS[ci]
                nc.sync.dma_start(wts[ci][:], wt[:, 4 * c0 : 4 * (c0 + cw)])

            dma_x(0, 0)
            dma_w(0)
            dma_x(1, 0)
            dma_w(1)
            dma_x(0, 1)
            dma_x(1, 1)
            for ci in range(2, N_CHUNKS):
                dma_w(ci)

            # ACT table warm-up: a tiny exp before any real work so the
            # ~2.7us PSEUDO_LOAD_ACT_FUNC_SET runs during the DMA fill.
            jt = xp.tile([P, 8], f32)
            nc.vector.memset(jt[:], 0.0)
            ja = xp.tile([P, 8], bf16)
            nc.scalar.activation(ja[:], jt[:], EXP)

            # PE warm-up: bridge from engine start to the first
            # data-dependent matmul so HAM un-throttles (~3.4us window).
            wsrc = xp.tile([P, MM_N], fp8, tag="warm_src")
            nc.vector.memset(wsrc[:], 1)
            for wi in range(N_WARM):
                pw = (psa_p if wi % 2 == 0 else psd_p).tile(
                    [P, HALF], f32, tag="ps", name="pw"
                )
                nc.tensor.matmul(
                    pw[:, :MM_N], wsrc[:, :P], wsrc[:], start=True, stop=True
                )

            # separate accumulators per engine; acc_a has 2 slots/unit
            # (the every-7th psd exp), zeroed once.
            acc_a = ac.tile([P, B_TILES, 2 * N_CHUNKS], f32)
            acc_d = ac.tile([P, B_TILES, N_CHUNKS], f32)
            nc.vector.memset(acc_a[:], 0.0)
            nc.vector.memset(acc_d[:], 0.0)
            out_sb = ac.tile([P, 3, B_TILES], f32)

            ui = 0
            for ci, cw in enumerate(CHUNKS):
                wt_t = wts[ci]
                wa = min(cw, HALF)          # psa columns
                wd = cw - wa                # psd columns
                for bt in range(B_TILES):
                    bh, bo = divmod(bt, 4)

                    def lhs(k):
                        return xt[(k, bh)][:, :, bo * P : (bo + 1) * P]

                    pa = psa_p.tile([P, HALF], f32, tag="ps", name="pa")
                    for si in range((wa + MM_N - 1) // MM_N):
                        s0 = si * MM_N
                        sw = min(MM_N, wa - s0)
                        for k in (0, 1):
                            nc.tensor.matmul(
                                pa[:, s0 : s0 + sw],
                                lhs(k),
                                wt_t[:, 2 * k : 2 * k + 2, s0 : s0 + sw],
                                start=(k == 0),
                                stop=(k == 1),
                                perf_mode=DR,
                            )
                    if wd:
                        pd = psd_p.tile([P, HALF], f32, tag="ps", name="pd")
                        for si in range((wd + MM_N - 1) // MM_N):
                            s0 = si * MM_N
                            sw = min(MM_N, wd - s0)
                            for k in (0, 1):
                                nc.tensor.matmul(
                                    pd[:, s0 : s0 + sw],
                                    lhs(k),
                                    wt_t[:, 2 * k : 2 * k + 2, wa + s0 : wa + s0 + sw],
                                    start=(k == 0),
                                    stop=(k == 1),
                                    perf_mode=DR,
                                )

                    # psa -> ACT exp + accumulate (value output is dead)
                    exd = ep.tile([P, HALF], bf16, tag="exd")
                    nc.scalar.activation(
                        exd[:, :wa],
                        pa[:, :wa],
                        EXP,
                        scale=ACT_SCALE,
                        accum_out=acc_a[:, bt, 2 * ci : 2 * ci + 1],
                    )
                    if wd:
                        if ui % ACT_PSD_EVERY == ACT_PSD_EVERY - 1:
                            # balance: ACT takes psd on this unit
                            exd2 = ep.tile([P, HALF], bf16, tag="exd")
                            nc.scalar.activation(
                                exd2[:, :wd],
                                pd[:, :wd],
                                EXP,
                                scale=ACT_SCALE,
                                accum_out=acc_a[:, bt, 2 * ci + 1 : 2 * ci + 2],
                            )
                        else:
                            # psd -> DVE Schraudolph + fused fold+accum
                            h = wd // 2
                            it = ip.tile([P, HALF], i16, tag="it")
                            nc.vector.tensor_scalar(
                                it[:, :wd],
                                pd[:, :wd],
                                SCH_A,
                                SCH_B,
                                mybir.AluOpType.mult,
                                mybir.AluOpType.add,
                            )
                            fo = fpool.tile([P, 512], bf16, tag="fo")
                            nc.vector.scalar_tensor_tensor(
                                fo[:, :h],
                                it[:, 0:h].bitcast(bf16),
                                1.0,
                                it[:, h:wd].bitcast(bf16),
                                mybir.AluOpType.mult,
                                mybir.AluOpType.add,
                                accum_out=acc_d[:, bt, ci : ci + 1],
                            )
                    ui += 1

            for bt in range(B_TILES):
                nc.vector.reduce_sum(
                    out_sb[:, 2, bt : bt + 1],
                    acc_d[:, bt, :],
                    axis=mybir.AxisListType.X,
                )
                nc.vector.reduce_sum(
                    out_sb[:, 0, bt : bt + 1],
                    acc_a[:, bt, 0 : 2 * N_CHUNKS : 2],
                    axis=mybir.AxisListType.X,
                )
                nc.vector.reduce_sum(
                    out_sb[:, 1, bt : bt + 1],
                    acc_a[:, bt, 1 : 2 * N_CHUNKS : 2],
                    axis=mybir.AxisListType.X,
                )
            nc.scalar.dma_start(out[:], out_sb[:])

    nc.compile()
    return nc


def _get_nc():
    global _NC_CACHE
    if _NC_CACHE is None:
        _NC_CACHE = _build_bass()
    return _NC_CACHE


def kernel(x: np.ndarray, labels: np.ndarray, W: np.ndarray) -> np.ndarray:
    global LAST_RESULT
    x = np.asarray(x, dtype=np.float32)
    W = np.asarray(W, dtype=np.float32)
    labels = np.asarray(labels)

    # ---- host prep (sharding glue) ----
    norms = np.maximum(np.sqrt((x.astype(np.float64) ** 2).sum(axis=1)), 1e-12)
    xn = (x / norms[:, None].astype(np.float32)).astype(np.float32)
    # xnt[p, ko, b] = xn[b, ko*128+p] * XSCALE
    xq = (
        np.ascontiguousarray(
            (xn.T * XSCALE).reshape(KO, P, B).transpose(1, 0, 2)
        )
        .astype(ml_dtypes.float8_e4m3)
        .reshape(P, KO * B)
    )

    in_maps = []
    for i in range(N_CORES):
        shard = W[i * C_SHARD : (i + 1) * C_SHARD]
        blocks = []
        c0 = 0
        for cw in CHUNKS:
            blk = (shard[c0 : c0 + cw].T * WSCALE).reshape(KO, P, cw)
            blocks.append(blk.transpose(1, 0, 2).reshape(P, KO * cw))
            c0 += cw
        wt_q = np.concatenate(blocks, axis=1).astype(ml_dtypes.float8_e4m3)
        in_maps.append({"xnt": xq, "wt": np.ascontiguousarray(wt_q)})

    # ---- device: per-core partial sum over classes of exp(s*logit) ----
    nc = _get_nc()
    res = run_bass_kernel_spmd(nc, in_maps, core_ids=list(range(N_CORES)))
    LAST_RESULT = res

    # ---- host combine (the all-reduce + tiny per-sample tail) ----
    sumexp = np.zeros(B, dtype=np.float64)
    for i in range(N_CORES):
        part = res.results[i]["out"].astype(np.float64)  # [P, 3, B_TILES]
        part = part.reshape(P, 3, B_TILES).sum(axis=1)   # [P, B_TILES]
        sumexp += part.T.reshape(B)                      # b = bt*128 + p

    target = np.einsum(
        "bd,bd->b", xn.astype(np.float64), W[labels].astype(np.float64)
    )
    tgt = np.clip(target, -1.0 + EPS, 1.0 - EPS)
    numerator = S_SCALE * np.cos(np.arccos(tgt) + MARGIN)
    excl = sumexp - np.exp(S_SCALE * tgt)
    L = numerator - np.log(np.exp(numerator) + excl)
    return np.array(-L.mean(), dtype=np.float32)


# revision 16
# speedup vs baseline: 1.3277x; 1.0347x over previous
"""ArcFace (AngularPenaltySMLoss) distributed Trainium2 kernel, v8.

Strategy (tensor-parallel over classes, per the sharding hint):
  - Shard W's C=100000 rows over 8 cores (12500 each).
  - Host: normalize x; pre-scale and cast x, W to fp8e4m3; lay both out
    chunk-contiguously so every DMA is 128 straight partition lines.
    All input DMAs ride ONE queue in need-order (x quarter tiles and
    the small first W chunk first) so nothing is starved.
  - Device (SPMD, no collectives): per (chunk, b-tile) unit, fp8
    DoubleRow matmuls fill TWO bank-aligned PSUM tiles: psa (cols
    0:1024) and psd (cols 1024:w). Tile/PSUM semantics serialize all
    consumers of one PSUM tile and make them wait for the whole fill,
    so each tile gets exactly ONE consumer:
      * psa -> ACT: exp(2*raw) + accum_out (free-dim sum straight into
        an accumulator slot; the exp value output goes to a dead SBUF
        dump tile).
      * psd -> DVE: Schraudolph bit-trick exp — tensor_scalar affine
        fp32->int16 (bits of bf16 exp), then one scalar_tensor_tensor
        fold-add over the bitcast-bf16 halves with accum_out. Every
        7th unit ACT takes psd instead (exp+accum) to balance engine
        load (~95% each, under the PE fill rate).
    ACT and DVE accumulate into SEPARATE tiles — a shared tile would
    serialize the two engines through Tile's write-order tracking.
  - Final per-bt reduce of each accumulator + [128, 24] DMA out; host
    adds the halves.
  - Host: sum partials over cores, compute the tiny per-sample target /
    arccos / log path in f64, return the scalar loss.
"""

import sys

if "/opt/trn_rl_repo" not in sys.path:
    sys.path.insert(0, "/opt/trn_rl_repo")

import ml_dtypes
import numpy as np

import concourse.bass as bass
import concourse.mybir as mybir
from concourse import bacc
from concourse.bass_utils import run_bass_kernel_spmd
from concourse.tile import TileContext

B, C, D = 1024, 100000, 512
S_SCALE, MARGIN, EPS = 64.0, 0.5, 1e-7
N_CORES = 8
C_SHARD = C // N_CORES          # 12500
P = 128
KO = D // P                     # 4 k-chunks of 128
B_TILES = B // P                # 8
HB = B // 2                     # x tile half-batch (512)
MM_N = 512                      # one matmul output <= one PSUM bank
HALF = 1024                     # psa width (2 PSUM banks)
N_WARM = 4                      # PE warm-up matmuls (bridge DMA fill + HAM)

WSCALE, XSCALE = 8.0, 4.0       # fp8 pre-scales (folded out via ACT_SCALE)
ACT_SCALE = S_SCALE / (WSCALE * XSCALE)   # 2.0

# Schraudolph bf16 exp bits: i16 = rint(A * raw + Badd); bitcast bf16.
SCH_A = ACT_SCALE * 184.66496580927726
SCH_B = 16256.0 - 7.4

CHUNKS = [512, 1748, 2048, 2048, 2048, 2048, 2048]
assert sum(CHUNKS) == C_SHARD
N_CHUNKS = len(CHUNKS)
ACT_PSD_EVERY = 7               # every 7th unit ACT consumes psd too

LAST_RESULT = None
_NC_CACHE = None


def _build_bass():
    nc = bacc.Bacc("TRN2")
    xnt = nc.declare_dram_parameter("xnt", [P, KO * B], mybir.dt.float8e4, isOutput=False)
    wt = nc.declare_dram_parameter("wt", [P, KO * C_SHARD], mybir.dt.float8e4, isOutput=False)
    out_a = nc.declare_dram_parameter(
        "out_a", [P, B_TILES * 2 * N_CHUNKS], mybir.dt.float32, isOutput=True
    )
    out_d = nc.declare_dram_parameter(
        "out_d", [P, B_TILES * N_CHUNKS], mybir.dt.float32, isOutput=True
    )

    fp8 = mybir.dt.float8e4
    f32 = mybir.dt.float32
    bf16 = mybir.dt.bfloat16
    i16 = mybir.dt.int16
    DR = mybir.MatmulPerfMode.DoubleRow
    EXP = mybir.ActivationFunctionType.Exp

    with TileContext(nc) as tc:
        with (
            tc.tile_pool(name="xp", bufs=1) as xp,
            tc.tile_pool(name="wp", bufs=1) as wp,
            tc.tile_pool(name="ip", bufs=3) as ip,
            tc.tile_pool(name="ep", bufs=3) as ep,
            tc.tile_pool(name="fp", bufs=2) as fpool,
            tc.tile_pool(name="ac", bufs=1) as ac,
            # one PSUM pool per consumer engine; single reader per tile
            tc.tile_pool(name="psa", bufs=2, space="PSUM") as psa_p,
            tc.tile_pool(name="psd", bufs=2, space="PSUM") as psd_p,
        ):
            # x quarter tiles (k-half x batch-half) + W chunks, all on one
            # queue ordered by first use. dram xnt layout: [p, ko, b].
            xt = {}
            for kh in (0, 1):
                for bh in (0, 1):
                    xt[(kh, bh)] = xp.tile(
                        [P, 2, HB], fp8, tag=f"x{kh}{bh}", name=f"x{kh}{bh}"
                    )

            def dma_x(kh, bh):
                src = xnt.rearrange("p (ko b) -> p ko b", ko=KO)[
                    :, 2 * kh : 2 * kh + 2, bh * HB : (bh + 1) * HB
                ]
                nc.sync.dma_start(xt[(kh, bh)][:], src)

            wts = []
            for ci, cw in enumerate(CHUNKS):
                wts.append(
                    wp.tile([P, KO, cw], fp8, tag=f"wt{ci}", name=f"wt{ci}")
                )

            def dma_w(ci):
                c0 = sum(CHUNKS[:ci])
                cw = CHUNKS[ci]
                nc.sync.dma_start(wts[ci][:], wt[:, 4 * c0 : 4 * (c0 + cw)])

            dma_x(0, 0)
            dma_w(0)
            dma_x(1, 0)
            dma_x(0, 1)
            dma_x(1, 1)
            for ci in range(1, N_CHUNKS):
                dma_w(ci)

            # ACT table warm-up: a tiny exp before any real work so the
            # ~2.7us PSEUDO_LOAD_ACT_FUNC_SET runs during the DMA fill.
            jt = xp.tile([P, 8], f32)
            nc.vector.memset(jt[:], 0.0)
            ja = xp.tile([P, 8], bf16)
            nc.scalar.activation(ja[:], jt[:], EXP)

            # PE warm-up: bridge from engine start to the first
            # data-dependent matmul so HAM un-throttles (~3.4us window).
            wsrc = xp.tile([P, MM_N], fp8, tag="warm_src")
            nc.vector.memset(wsrc[:], 1)
            for wi in range(N_WARM):
                pw = (psa_p if wi % 2 == 0 else psd_p).tile(
                    [P, HALF], f32, tag="ps", name="pw"
                )
                nc.tensor.matmul(
                    pw[:, :MM_N], wsrc[:, :P], wsrc[:], start=True, stop=True
                )

            # separate accumulators per engine; acc_a has 2 slots/unit
            # (the every-7th psd exp), zeroed once.
            acc_a = ac.tile([P, B_TILES, 2 * N_CHUNKS], f32)
            acc_d = ac.tile([P, B_TILES, N_CHUNKS], f32)
            nc.vector.memset(acc_a[:], 0.0)
            nc.vector.memset(acc_d[:], 0.0)

            ui = 0
            for ci, cw in enumerate(CHUNKS):
                wt_t = wts[ci]
                wa = min(cw, HALF)          # psa columns
                wd = cw - wa                # psd columns
                for bt in range(B_TILES):
                    bh, bo = divmod(bt, 4)

                    def lhs(k):
                        return xt[(k, bh)][:, :, bo * P : (bo + 1) * P]

                    pa = psa_p.tile([P, HALF], f32, tag="ps", name="pa")
                    for si in range((wa + MM_N - 1) // MM_N):
                        s0 = si * MM_N
                        sw = min(MM_N, wa - s0)
                        for k in (0, 1):
                            nc.tensor.matmul(
                                pa[:, s0 : s0 + sw],
                                lhs(k),
                                wt_t[:, 2 * k : 2 * k + 2, s0 : s0 + sw],
                                start=(k == 0),
                                stop=(k == 1),
                                perf_mode=DR,
                            )
                    if wd:
                        pd = psd_p.tile([P, HALF], f32, tag="ps", name="pd")
                        for si in range((wd + MM_N - 1) // MM_N):
                            s0 = si * MM_N
                            sw = min(MM_N, wd - s0)
                            for k in (0, 1):
                                nc.tensor.matmul(
                                    pd[:, s0 : s0 + sw],
                                    lhs(k),
                                    wt_t[:, 2 * k : 2 * k + 2, wa + s0 : wa + s0 + sw],
                                    start=(k == 0),
                                    stop=(k == 1),
                                    perf_mode=DR,
                                )

                    # psa -> ACT exp + accumulate (value output is dead)
                    exd = ep.tile([P, HALF], bf16, tag="exd")
                    nc.scalar.activation(
                        exd[:, :wa],
                        pa[:, :wa],
                        EXP,
                        scale=ACT_SCALE,
                        accum_out=acc_a[:, bt, 2 * ci : 2 * ci + 1],
                    )
                    if wd:
                        if ui % ACT_PSD_EVERY == ACT_PSD_EVERY - 1:
                            # balance: ACT takes psd on this unit
                            exd2 = ep.tile([P, HALF], bf16, tag="exd")
                            nc.scalar.activation(
                                exd2[:, :wd],
                                pd[:, :wd],
                                EXP,
                                scale=ACT_SCALE,
                                accum_out=acc_a[:, bt, 2 * ci + 1 : 2 * ci + 2],
                            )
                        else:
                            # psd -> DVE Schraudolph + fused fold+accum
                            h = wd // 2
                            it = ip.tile([P, HALF], i16, tag="it")
                            nc.vector.tensor_scalar(
                                it[:, :wd],
                                pd[:, :wd],
                                SCH_A,
                                SCH_B,
                                mybir.AluOpType.mult,
                                mybir.AluOpType.add,
                            )
                            fo = fpool.tile([P, 512], bf16, tag="fo")
                            nc.vector.scalar_tensor_tensor(
                                fo[:, :h],
                                it[:, 0:h].bitcast(bf16),
                                1.0,
                                it[:, h:wd].bitcast(bf16),
                                mybir.AluOpType.mult,
                                mybir.AluOpType.add,
                                accum_out=acc_d[:, bt, ci : ci + 1],
                            )
                    ui += 1

            # ship raw accumulator slots; the host does the tiny final sum
            nc.scalar.dma_start(out_a[:], acc_a[:])
            nc.sync.dma_start(out_d[:], acc_d[:])

    nc.compile()
    return nc


def _get_nc():
    global _NC_CACHE
    if _NC_CACHE is None:
        _NC_CACHE = _build_bass()
    return _NC_CACHE


def kernel(x: np.ndarray, labels: np.ndarray, W: np.ndarray) -> np.ndarray:
    global LAST_RESULT
    x = np.asarray(x, dtype=np.float32)
    W = np.asarray(W, dtype=np.float32)
    labels = np.asarray(labels)

    # ---- host prep (sharding glue) ----
    norms = np.maximum(np.sqrt((x.astype(np.float64) ** 2).sum(axis=1)), 1e-12)
    xn = (x / norms[:, None].astype(np.float32)).astype(np.float32)
    # xnt[p, ko, b] = xn[b, ko*128+p] * XSCALE
    xq = (
        np.ascontiguousarray(
            (xn.T * XSCALE).reshape(KO, P, B).transpose(1, 0, 2)
        )
        .astype(ml_dtypes.float8_e4m3)
        .reshape(P, KO * B)
    )

    in_maps = []
    for i in range(N_CORES):
        shard = W[i * C_SHARD : (i + 1) * C_SHARD]
        blocks = []
        c0 = 0
        for cw in CHUNKS:
            blk = (shard[c0 : c0 + cw].T * WSCALE).reshape(KO, P, cw)
            blocks.append(blk.transpose(1, 0, 2).reshape(P, KO * cw))
            c0 += cw
        wt_q = np.concatenate(blocks, axis=1).astype(ml_dtypes.float8_e4m3)
        in_maps.append({"xnt": xq, "wt": np.ascontiguousarray(wt_q)})

    # ---- device: per-core partial sum over classes of exp(s*logit) ----
    nc = _get_nc()
    res = run_bass_kernel_spmd(nc, in_maps, core_ids=list(range(N_CORES)))
    LAST_RESULT = res

    # ---- host combine (the all-reduce + tiny per-sample tail) ----
    sumexp = np.zeros(B, dtype=np.float64)
    for i in range(N_CORES):
        pa = res.results[i]["out_a"].astype(np.float64)
        pd = res.results[i]["out_d"].astype(np.float64)
        part = (
            pa.reshape(P, B_TILES, 2 * N_CHUNKS).sum(axis=2)
            + pd.reshape(P, B_TILES, N_CHUNKS).sum(axis=2)
        )                                                # [P, B_TILES]
        sumexp += part.T.reshape(B)                      # b = bt*128 + p

    target = np.einsum(
        "bd,bd->b", xn.astype(np.float64), W[labels].astype(np.float64)
    )
    tgt = np.clip(target, -1.0 + EPS, 1.0 - EPS)
    numerator = S_SCALE * np.cos(np.arccos(tgt) + MARGIN)
    excl = sumexp - np.exp(S_SCALE * tgt)
    L = numerator - np.log(np.exp(numerator) + excl)
    return np.array(-L.mean(), dtype=np.float32)


# revision 17
# speedup vs baseline: 1.3458x; 1.0136x over previous
"""ArcFace (AngularPenaltySMLoss) distributed Trainium2 kernel, v8.

Strategy (tensor-parallel over classes, per the sharding hint):
  - Shard W's C=100000 rows over 8 cores (12500 each).
  - Host: normalize x; pre-scale and cast x, W to fp8e4m3; lay both out
    chunk-contiguously so every DMA is 128 straight partition lines.
    All input DMAs ride ONE queue in need-order (x quarter tiles and
    the small first W chunk first) so nothing is starved.
  - Device (SPMD, no collectives): per (chunk, b-tile) unit, fp8
    DoubleRow matmuls fill TWO bank-aligned PSUM tiles: psa (cols
    0:1024) and psd (cols 1024:w). Tile/PSUM semantics serialize all
    consumers of one PSUM tile and make them wait for the whole fill,
    so each tile gets exactly ONE consumer:
      * psa -> ACT: exp(2*raw) + accum_out (free-dim sum straight into
        an accumulator slot; the exp value output goes to a dead SBUF
        dump tile).
      * psd -> DVE: Schraudolph bit-trick exp — tensor_scalar affine
        fp32->int16 (bits of bf16 exp), then one scalar_tensor_tensor
        fold-add over the bitcast-bf16 halves with accum_out. Every
        7th unit ACT takes psd instead (exp+accum) to balance engine
        load (~95% each, under the PE fill rate).
    ACT and DVE accumulate into SEPARATE tiles — a shared tile would
    serialize the two engines through Tile's write-order tracking.
  - Final per-bt reduce of each accumulator + [128, 24] DMA out; host
    adds the halves.
  - Host: sum partials over cores, compute the tiny per-sample target /
    arccos / log path in f64, return the scalar loss.
"""

import sys

if "/opt/trn_rl_repo" not in sys.path:
    sys.path.insert(0, "/opt/trn_rl_repo")

import ml_dtypes
import numpy as np

import concourse.bass as bass
import concourse.mybir as mybir
from concourse import bacc
from concourse.bass_utils import run_bass_kernel_spmd
from concourse.tile import TileContext

B, C, D = 1024, 100000, 512
S_SCALE, MARGIN, EPS = 64.0, 0.5, 1e-7
N_CORES = 8
C_SHARD = C // N_CORES          # 12500
P = 128
KO = D // P                     # 4 k-chunks of 128
B_TILES = B // P                # 8
HB = B // 2                     # x tile half-batch (512)
MM_N = 512                      # one matmul output <= one PSUM bank
HALF = 1024                     # psa width (2 PSUM banks)
N_WARM = 4                      # PE warm-up matmuls (bridge DMA fill + HAM)

WSCALE, XSCALE = 8.0, 4.0       # fp8 pre-scales (folded out via ACT_SCALE)
ACT_SCALE = S_SCALE / (WSCALE * XSCALE)   # 2.0

# Schraudolph bf16 exp bits: i16 = rint(A * raw + Badd); bitcast bf16.
SCH_A = ACT_SCALE * 184.66496580927726
SCH_B = 16256.0 - 7.4

CHUNKS = [512, 1748, 2048, 2048, 2048, 2048, 2048]
assert sum(CHUNKS) == C_SHARD
N_CHUNKS = len(CHUNKS)
ACT_PSD_EVERY = 7               # every 7th unit ACT consumes psd too

LAST_RESULT = None
_NC_CACHE = None


def _build_bass():
    nc = bacc.Bacc("TRN2")
    xnt = nc.declare_dram_parameter("xnt", [P, KO * B], mybir.dt.float8e4, isOutput=False)
    wt = nc.declare_dram_parameter("wt", [P, KO * C_SHARD], mybir.dt.float8e4, isOutput=False)
    out_a = nc.declare_dram_parameter(
        "out_a", [P, B_TILES * 2 * N_CHUNKS], mybir.dt.float32, isOutput=True
    )
    out_d = nc.declare_dram_parameter(
        "out_d", [P, B_TILES * N_CHUNKS], mybir.dt.float32, isOutput=True
    )

    fp8 = mybir.dt.float8e4
    f32 = mybir.dt.float32
    bf16 = mybir.dt.bfloat16
    i16 = mybir.dt.int16
    DR = mybir.MatmulPerfMode.DoubleRow
    EXP = mybir.ActivationFunctionType.Exp

    with TileContext(nc) as tc:
        with (
            tc.tile_pool(name="xp", bufs=1) as xp,
            tc.tile_pool(name="wp", bufs=1) as wp,
            tc.tile_pool(name="ip", bufs=3) as ip,
            tc.tile_pool(name="ep", bufs=3) as ep,
            tc.tile_pool(name="fp", bufs=2) as fpool,
            tc.tile_pool(name="ac", bufs=1) as ac,
            # one PSUM pool per consumer engine; single reader per tile
            tc.tile_pool(name="psa", bufs=2, space="PSUM") as psa_p,
            tc.tile_pool(name="psd", bufs=2, space="PSUM") as psd_p,
        ):
            # x quarter tiles (k-half x batch-half) + W chunks, all on one
            # queue ordered by first use. dram xnt layout: [p, ko, b].
            xt = {}
            for kh in (0, 1):
                for bh in (0, 1):
                    xt[(kh, bh)] = xp.tile(
                        [P, 2, HB], fp8, tag=f"x{kh}{bh}", name=f"x{kh}{bh}"
                    )

            def dma_x(kh, bh):
                src = xnt.rearrange("p (ko b) -> p ko b", ko=KO)[
                    :, 2 * kh : 2 * kh + 2, bh * HB : (bh + 1) * HB
                ]
                nc.sync.dma_start(xt[(kh, bh)][:], src)

            wts = []
            for ci, cw in enumerate(CHUNKS):
                wts.append(
                    wp.tile([P, KO, cw], fp8, tag=f"wt{ci}", name=f"wt{ci}")
                )

            def dma_w(ci):
                c0 = sum(CHUNKS[:ci])
                cw = CHUNKS[ci]
                nc.sync.dma_start(wts[ci][:], wt[:, 4 * c0 : 4 * (c0 + cw)])

            dma_x(0, 0)
            dma_w(0)
            dma_x(1, 0)
            dma_x(0, 1)
            dma_x(1, 1)
            for ci in range(1, N_CHUNKS):
                dma_w(ci)

            # ACT table warm-up: a tiny exp before any real work so the
            # ~2.7us PSEUDO_LOAD_ACT_FUNC_SET runs during the DMA fill.
            jt = xp.tile([P, 8], f32)
            nc.vector.memset(jt[:], 0.0)
            ja = xp.tile([P, 8], bf16)
            nc.scalar.activation(ja[:], jt[:], EXP)

            # PE warm-up: bridge from engine start to the first
            # data-dependent matmul so HAM un-throttles (~3.4us window).
            wsrc = xp.tile([P, MM_N], fp8, tag="warm_src")
            nc.vector.memset(wsrc[:], 1)
            for wi in range(N_WARM):
                pw = (psa_p if wi % 2 == 0 else psd_p).tile(
                    [P, HALF], f32, tag="ps", name="pw"
                )
                nc.tensor.matmul(
                    pw[:, :MM_N], wsrc[:, :P], wsrc[:], start=True, stop=True
                )

            # separate accumulators per engine; acc_a has 2 slots/unit
            # (the every-7th psd exp), zeroed once.
            acc_a = ac.tile([P, B_TILES, 2 * N_CHUNKS], f32)
            acc_d = ac.tile([P, B_TILES, N_CHUNKS], f32)
            nc.vector.memset(acc_a[:], 0.0)
            nc.vector.memset(acc_d[:], 0.0)

            ui = 0
            for ci, cw in enumerate(CHUNKS):
                wt_t = wts[ci]
                wa = min(cw, HALF)          # psa columns
                wd = cw - wa                # psd columns
                for bt in range(B_TILES):
                    if ci == 0 and bt == 4:
                        # bridge matmuls: keep the PE (and HAM's activity
                        # window) busy while the second batch-half x tiles
                        # finish their DMA.
                        for wi in range(3):
                            pw2 = (psa_p if wi % 2 == 0 else psd_p).tile(
                                [P, HALF], f32, tag="ps", name="pw2"
                            )
                            nc.tensor.matmul(
                                pw2[:, :MM_N], wsrc[:, :P], wsrc[:],
                                start=True, stop=True,
                            )
                    bh, bo = divmod(bt, 4)

                    def lhs(k):
                        return xt[(k, bh)][:, :, bo * P : (bo + 1) * P]

                    pa = psa_p.tile([P, HALF], f32, tag="ps", name="pa")
                    for si in range((wa + MM_N - 1) // MM_N):
                        s0 = si * MM_N
                        sw = min(MM_N, wa - s0)
                        for k in (0, 1):
                            nc.tensor.matmul(
                                pa[:, s0 : s0 + sw],
                                lhs(k),
                                wt_t[:, 2 * k : 2 * k + 2, s0 : s0 + sw],
                                start=(k == 0),
                                stop=(k == 1),
                                perf_mode=DR,
                            )
                    if wd:
                        pd = psd_p.tile([P, HALF], f32, tag="ps", name="pd")
                        for si in range((wd + MM_N - 1) // MM_N):
                            s0 = si * MM_N
                            sw = min(MM_N, wd - s0)
                            for k in (0, 1):
                                nc.tensor.matmul(
                                    pd[:, s0 : s0 + sw],
                                    lhs(k),
                                    wt_t[:, 2 * k : 2 * k + 2, wa + s0 : wa + s0 + sw],
                                    start=(k == 0),
                                    stop=(k == 1),
                                    perf_mode=DR,
                                )

                    # psa -> ACT exp + accumulate (value output is dead)
                    exd = ep.tile([P, HALF], bf16, tag="exd")
                    nc.scalar.activation(
                        exd[:, :wa],
                        pa[:, :wa],
                        EXP,
                        scale=ACT_SCALE,
                        accum_out=acc_a[:, bt, 2 * ci : 2 * ci + 1],
                    )
                    if wd:
                        if wd == HALF and ui % ACT_PSD_EVERY == ACT_PSD_EVERY - 1:
                            # balance: ACT takes psd on this unit
                            exd2 = ep.tile([P, HALF], bf16, tag="exd")
                            nc.scalar.activation(
                                exd2[:, :wd],
                                pd[:, :wd],
                                EXP,
                                scale=ACT_SCALE,
                                accum_out=acc_a[:, bt, 2 * ci + 1 : 2 * ci + 2],
                            )
                        else:
                            # psd -> DVE Schraudolph + fused fold+accum
                            h = wd // 2
                            it = ip.tile([P, HALF], i16, tag="it")
                            nc.vector.tensor_scalar(
                                it[:, :wd],
                                pd[:, :wd],
                                SCH_A,
                                SCH_B,
                                mybir.AluOpType.mult,
                                mybir.AluOpType.add,
                            )
                            fo = fpool.tile([P, 512], bf16, tag="fo")
                            nc.vector.scalar_tensor_tensor(
                                fo[:, :h],
                                it[:, 0:h].bitcast(bf16),
                                1.0,
                                it[:, h:wd].bitcast(bf16),
                                mybir.AluOpType.mult,
                                mybir.AluOpType.add,
                                accum_out=acc_d[:, bt, ci : ci + 1],
                            )
                    ui += 1

            # ship raw accumulator slots; the host does the tiny final sum
            nc.scalar.dma_start(out_a[:], acc_a[:])
            nc.sync.dma_start(out_d[:], acc_d[:])

    nc.compile()
    return nc


def _get_nc():
    global _NC_CACHE
    if _NC_CACHE is None:
        _NC_CACHE = _build_bass()
    return _NC_CACHE


def kernel(x: np.ndarray, labels: np.ndarray, W: np.ndarray) -> np.ndarray:
    global LAST_RESULT
    x = np.asarray(x, dtype=np.float32)
    W = np.asarray(W, dtype=np.float32)
    labels = np.asarray(labels)

    # ---- host prep (sharding glue) ----
    norms = np.maximum(np.sqrt((x.astype(np.float64) ** 2).sum(axis=1)), 1e-12)
    xn = (x / norms[:, None].astype(np.float32)).astype(np.float32)
    # xnt[p, ko, b] = xn[b, ko*128+p] * XSCALE
    xq = (
        np.ascontiguousarray(
            (xn.T * XSCALE).reshape(KO, P, B).transpose(1, 0, 2)
        )
        .astype(ml_dtypes.float8_e4m3)
        .reshape(P, KO * B)
    )

    in_maps = []
    for i in range(N_CORES):
        shard = W[i * C_SHARD : (i + 1) * C_SHARD]
        blocks = []
        c0 = 0
        for cw in CHUNKS:
            blk = (shard[c0 : c0 + cw].T * WSCALE).reshape(KO, P, cw)
            blocks.append(blk.transpose(1, 0, 2).reshape(P, KO * cw))
            c0 += cw
        wt_q = np.concatenate(blocks, axis=1).astype(ml_dtypes.float8_e4m3)
        in_maps.append({"xnt": xq, "wt": np.ascontiguousarray(wt_q)})

    # ---- device: per-core partial sum over classes of exp(s*logit) ----
    nc = _get_nc()
    res = run_bass_kernel_spmd(nc, in_maps, core_ids=list(range(N_CORES)))
    LAST_RESULT = res

    # ---- host combine (the all-reduce + tiny per-sample tail) ----
    sumexp = np.zeros(B, dtype=np.float64)
    for i in range(N_CORES):
        pa = res.results[i]["out_a"].astype(np.float64)
        pd = res.results[i]["out_d"].astype(np.float64)
        part = (
            pa.reshape(P, B_TILES, 2 * N_CHUNKS).sum(axis=2)
            + pd.reshape(P, B_TILES, N_CHUNKS).sum(axis=2)
        )                                                # [P, B_TILES]
        sumexp += part.T.reshape(B)                      # b = bt*128 + p

    target = np.einsum(
        "bd,bd->b", xn.astype(np.float64), W[labels].astype(np.float64)
    )
    tgt = np.clip(target, -1.0 + EPS, 1.0 - EPS)
    numerator = S_SCALE * np.cos(np.arccos(tgt) + MARGIN)
    excl = sumexp - np.exp(S_SCALE * tgt)
    L = numerator - np.log(np.exp(numerator) + excl)
    return np.array(-L.mean(), dtype=np.float32)
